# revision 18
# baseline (speedup 1.0000x reference)
"""Trainium2 Bass kernel for grouped-query attention with qk-norm.

Problem (hardcoded): x(2,2048,1024) @ Wq(1024,1024) / Wkv(1024,512),
16 query heads, 4 kv heads, head_dim 64, k_scale(16,1,64) applied to the
group-broadcast k. Output (2,2048,1024).

Sharding: 8 cores = batch(2) x kv_heads(4). Each core computes its batch's
4 query heads against its kv head over the full 2048x2048 score matrix.

Device kernel layout choices:
- Host passes x transposed (xT: dim on partitions) so all projection
  matmuls contract over dim with no on-device transposes.
- k_scale is folded into Wk host-side: (x@Wk)*ks == x@(Wk*diag(ks)),
  giving a per-query-head scaled kkT directly from the projection.
- Scores are computed transposed (S^T: keys on partitions, queries free)
  so that exp(S^T) tiles feed the PV matmul directly as the moving
  operand (no P transpose).
- Softmax skips the max-subtraction (inputs are bounded; exp stays well
  inside fp32 range) and normalizes after PV via an appended ones-row in
  the V stationary operand (row 64 of the PV psum accumulates sum(exp)).
- Output is returned transposed per head (oT: 4*64 x 2048); the host
  transposes during the gather.
- Matmul inputs are float32r (fp32 storage, reduced-precision multiply,
  4x the fp32 throughput at moving-dim >= 256).
"""

import os
from contextlib import ExitStack

import numpy as np

import concourse.bacc as bacc
import concourse.mybir as mybir
import concourse.tile as tile
from concourse.bass_utils import run_bass_kernel_spmd

# Problem constants
B, N, DIM = 2, 2048, 1024
HEADS, KV_HEADS, DH = 16, 4, 64
G = HEADS // KV_HEADS  # query heads per kv head (4)
NCORES = 8
P = 128
KT = DIM // P  # 8 contraction tiles over dim
IC = 512  # query-chunk width
NI = N // IC  # 4
NJ = N // P  # 16 key tiles
SCALE = DH**-0.5

F32 = mybir.dt.float32
F32R = mybir.dt.float32r

# matmul input dtype: fp32r streams 1 row/cycle at N>=256 (4x faster than fp32)
USE_F32R = os.environ.get("KERNEL_F32", "0") != "1"
DMM = F32R if USE_F32R else F32


def emit_kernel(ctx, tc, xT, wq, wk, wv, oT):
    nc = tc.nc
    Exp = mybir.ActivationFunctionType.Exp
    mult = mybir.AluOpType.mult

    def dr(ap):  # dram-side view matching the matmul dtype
        return ap.bitcast(DMM) if USE_F32R else ap

    wpool = ctx.enter_context(tc.tile_pool(name="w", bufs=1))
    qkpool = ctx.enter_context(tc.tile_pool(name="qk", bufs=1))

    # --- persistent SBUF tensors ---
    ones_sb = wpool.tile([P, DH], DMM, tag="ones")  # 1-row slices as bc lhsT
    qT = [qkpool.tile([P, N], DMM, name=f"qT{hp}", tag=f"qT{hp}") for hp in range(2)]
    kkT = [qkpool.tile([P, N], DMM, name=f"kkT{hp}", tag=f"kkT{hp}") for hp in range(2)]
    vaug = qkpool.tile([P, NJ * (DH + 1)], DMM, tag="vaug")  # (128, 16*65)
    nc.any.memset(vaug[:].bitcast(F32), 1.0)
    nc.any.memset(ones_sb[:].bitcast(F32), 1.0)
    warm = qkpool.tile([1, 1], F32, tag="warm")
    nc.scalar.activation(warm[:], ones_sb[0:1, 0:1].bitcast(F32), Exp)

    # --- projections: qT / kkT (d on partitions, tokens free) ---
    # xts + weights live only for this phase; the pool is released after.
    with tc.tile_pool(name="xw", bufs=1) as xwpool:
        wq_sb = xwpool.tile([P, KT * 256], DMM, tag="wq")  # (128, 2048)
        wk_sb = xwpool.tile([P, KT * 256], DMM, tag="wk")
        wv_sb = xwpool.tile([P, KT * DH], DMM, tag="wv")  # (128, 512)
        xts = xwpool.tile([P, KT * N], DMM, tag="xt")  # (128, 16384) = 8MB
        def dma_x(kt, ic):
            r = slice(kt * P, (kt + 1) * P)
            csl = slice(ic * IC, (ic + 1) * IC)
            nc.sync.dma_start(
                xts[:, kt * N + ic * IC : kt * N + (ic + 1) * IC], dr(xT[r, csl])
            )

        for kt in range(KT):
            r = slice(kt * P, (kt + 1) * P)
            nc.sync.dma_start(wq_sb[:, kt * 256 : (kt + 1) * 256], dr(wq[r, :]))
            dma_x(kt, 0)
        for kt in range(KT):
            r = slice(kt * P, (kt + 1) * P)
            nc.sync.dma_start(wk_sb[:, kt * 256 : (kt + 1) * 256], dr(wk[r, :]))
            dma_x(kt, 1)
        for kt in range(KT):
            r = slice(kt * P, (kt + 1) * P)
            nc.sync.dma_start(wv_sb[:, kt * DH : (kt + 1) * DH], dr(wv[r, :]))
            dma_x(kt, 2)
        for kt in range(KT):
            dma_x(kt, 3)

        with tc.tile_pool(name="pp", bufs=4, space="PSUM") as pp:
            for hp in range(2):
                for t, w_sb in ((qT[hp], wq_sb), (kkT[hp], wk_sb)):
                    for ic in range(NI):
                        ps = pp.tile([P, IC], F32, tag="pj")
                        for kt in range(KT):
                            c0 = kt * 256 + hp * 128
                            nc.tensor.matmul(
                                ps[:],
                                w_sb[:, c0 : c0 + 128],
                                xts[:, kt * N + ic * IC : kt * N + (ic + 1) * IC],
                                start=(kt == 0),
                                stop=(kt == KT - 1),
                            )
                        nc.vector.tensor_copy(t[:, ic * IC : (ic + 1) * IC], ps[:])
            # v in natural layout (tokens on partitions, d free), interleaved
            # with a ones column per key-tile for the sum(exp) row of PV.
            for jt in range(NJ):
                ps = pp.tile([P, DH], F32, tag="vp", bufs=2)
                for kt in range(KT):
                    nc.tensor.matmul(
                        ps[:],
                        xts[:, kt * N + jt * P : kt * N + (jt + 1) * P],
                        wv_sb[:, kt * DH : (kt + 1) * DH],
                        start=(kt == 0),
                        stop=(kt == KT - 1),
                    )
                nc.vector.tensor_copy(
                    vaug[:, jt * (DH + 1) : jt * (DH + 1) + DH], ps[:]
                )

    ptpool = ctx.enter_context(tc.tile_pool(name="pt", bufs=4))
    npool = ctx.enter_context(tc.tile_pool(name="norm", bufs=2))
    sums_d = nc.dram_tensor("sums_d", (G, N), F32, kind="ExternalOutput").ap()
    rec_d = nc.dram_tensor("rec_d", (G, N), F32, kind="ExternalOutput").ap()

    # --- attention ---
    # Hot loop emits only matmuls + exp + a psum drain copy; normalization
    # is deferred per head so the PE/ACT pipeline never stalls on it.
    o_acc = [
        npool.tile([DH + 1, N], F32, name=f"oacc{h}", tag=f"oacc{h}", bufs=1)
        for h in range(G)
    ]

    # One shared tile holds the 4 heads' f32r reciprocal rows at
    # partitions 0/32/64/96 (matching the bc-matmul tile_position rows).
    recr = npool.tile([97, N], DMM, tag="recr", bufs=1)
    # DVE reciprocal on a 1-row (1,2048) AP costs ~13us; on (128,16) it is
    # ~100x cheaper. The sums row is respread across partitions via a DRAM
    # bounce (DMA cannot repartition within SBUF).

    def recip_chunk(h, ic):
        csl = slice(ic * IC, (ic + 1) * IC)
        sums_t = npool.tile([P, 4], F32, tag="sums_t", bufs=2)
        rec_t = npool.tile([P, 4], F32, tag="rec_t", bufs=2)
        nc.sync.dma_start(
            sums_t[:], sums_d[h : h + 1, csl].rearrange("o (p f) -> (o p) f", p=P)
        )
        nc.vector.reciprocal(rec_t[:], sums_t[:])
        nc.sync.dma_start(
            rec_d[h : h + 1, csl].rearrange("o (p f) -> (o p) f", p=P), rec_t[:]
        )
        nc.sync.dma_start(recr[32 * h : 32 * h + 1, csl], dr(rec_d[h : h + 1, csl]))

    def normalize_head(h, apsum):
        for ic in range(NI):
            csl = slice(ic * IC, (ic + 1) * IC)
            bc = apsum.tile([DH, IC], F32, name="bc", tag="s", bufs=3)
            nc.tensor.matmul(
                bc[:],
                ones_sb[32 * h : 32 * h + 1, :],
                recr[32 * h : 32 * h + 1, csl],
                start=True,
                stop=True,
                tile_position=(32 * h, 0),
            )
            fin = npool.tile([DH, IC], F32, tag="fin", bufs=4)
            nc.vector.tensor_tensor(fin[:], o_acc[h][0:DH, csl], bc[:], mult)
            nc.sync.dma_start(oT[h * DH : (h + 1) * DH, csl], fin[:])

    with tc.tile_pool(name="ap", bufs=3, space="PSUM") as apsum:
        for hp in range(2):
            for ic in range(NI):
                o_ps = [
                    apsum.tile([DH + 1, IC], F32, name=f"ops{i}", tag=f"ops{i}", bufs=1)
                    for i in range(2)
                ]
                for jt in range(NJ):
                    s = apsum.tile([P, 2 * IC], F32, tag="s", bufs=3)
                    for half in range(2):
                        rsl = slice(half * 64, half * 64 + 64)
                        nc.tensor.matmul(
                            s[:, half * IC : (half + 1) * IC],
                            kkT[hp][rsl, jt * P : (jt + 1) * P],
                            qT[hp][rsl, ic * IC : (ic + 1) * IC],
                            start=True,
                            stop=True,
                            tile_position=(half * 64, 0),
                        )
                    pt = ptpool.tile([P, 2 * IC], DMM, tag="pt")
                    nc.scalar.activation(pt[:], s[:], Exp, scale=SCALE)
                    for half in range(2):
                        nc.tensor.matmul(
                            o_ps[half][:],
                            vaug[:, jt * (DH + 1) : (jt + 1) * (DH + 1)],
                            pt[:, half * IC : (half + 1) * IC],
                            start=(jt == 0),
                            stop=(jt == NJ - 1),
                        )
                for half in range(2):
                    h = 2 * hp + half
                    nc.vector.tensor_copy(
                        o_acc[h][:, ic * IC : (ic + 1) * IC], o_ps[half][:]
                    )
                    nc.sync.dma_start(
                        sums_d[h : h + 1, ic * IC : (ic + 1) * IC],
                        o_acc[h][DH : DH + 1, ic * IC : (ic + 1) * IC],
                    )
                    recip_chunk(h, ic)
        for h in range(G):
            normalize_head(h, apsum)


_CACHE = {}


def build():
    if "nc" in _CACHE:
        return _CACHE["nc"]
    nc = bacc.Bacc(
        "TRN2", target_bir_lowering=False, debug=False, num_devices=NCORES
    )
    xT = nc.dram_tensor("xT", (DIM, N), F32, kind="ExternalInput").ap()
    wq = nc.dram_tensor("wq", (DIM, G * DH), F32, kind="ExternalInput").ap()
    wk = nc.dram_tensor("wk", (DIM, G * DH), F32, kind="ExternalInput").ap()
    wv = nc.dram_tensor("wv", (DIM, DH), F32, kind="ExternalInput").ap()
    oT = nc.dram_tensor("oT", (G * DH, N), F32, kind="ExternalOutput").ap()
    with tile.TileContext(nc) as tc:
        with ExitStack() as ctx:
            emit_kernel(ctx, tc, xT, wq, wk, wv, oT)
    nc.compile()
    _CACHE["nc"] = nc
    return nc


def make_in_maps(x, Wq, Wkv, k_scale):
    x = np.asarray(x, dtype=np.float32)
    Wq = np.asarray(Wq, dtype=np.float32)
    Wkv = np.asarray(Wkv, dtype=np.float32)
    k_scale = np.asarray(k_scale, dtype=np.float32)
    xTs = [np.ascontiguousarray(x[b].T) for b in range(B)]
    in_maps = []
    for c in range(NCORES):
        b, kv = divmod(c, KV_HEADS)
        wk_base = Wkv[:, kv * DH : (kv + 1) * DH]
        wk_c = np.concatenate(
            [wk_base * k_scale[kv * G + j, 0][None, :] for j in range(G)], axis=1
        )
        in_maps.append(
            {
                "xT": xTs[b],
                "wq": np.ascontiguousarray(Wq[:, kv * G * DH : (kv + 1) * G * DH]),
                "wk": np.ascontiguousarray(wk_c),
                "wv": np.ascontiguousarray(
                    Wkv[:, KV_HEADS * DH + kv * DH : KV_HEADS * DH + (kv + 1) * DH]
                ),
            }
        )
    return in_maps


def gather(results):
    out = np.empty((B, N, HEADS * DH), dtype=np.float32)
    for c in range(NCORES):
        b, kv = divmod(c, KV_HEADS)
        out[b, :, kv * G * DH : (kv + 1) * G * DH] = results[c]["oT"].T
    return out


def kernel(x, Wq, Wkv, k_scale, _trace=False):
    nc = build()
    in_maps = make_in_maps(x, Wq, Wkv, k_scale)
    res = run_bass_kernel_spmd(
        nc, in_maps, core_ids=list(range(NCORES)), trace=_trace
    )
    out = gather(res.results)
    if _trace:
        kernel.last_result = res
    return out
